# revision 14
# baseline (speedup 1.0000x reference)
"""LoRA QKV projection kernel for 8 Trainium2 NeuronCores.

Reference computation (per problem):
    qkv = x @ Wqkv^T + bqkv + concat(x@Aq^T@Bq^T, x@Ak^T@Bk^T, x@Av^T@Bv^T)

Strategy:
  * Host folds the rank-16 LoRA factors into the dense weight
    (W_eff = Wqkv + blockdiag(BqAq, BkAk, BvAv) — ~56 MFLOP, 0.05% of the
    116 GFLOP GEMM), so the device runs one pure GEMM at the roofline ridge.
  * Data-parallel: batch dim (8) sharded 1:1 over the 8 cores.
    Each core: y[4096, 2304] = x_b[4096, 768] @ W_eff^T.
  * bf16 everywhere off-chip: x and W_eff are cast to bf16 on host (halves
    input DMA, enables fast-weight-load on the PE), the GEMM accumulates in
    fp32 PSUM, and the output is stored as bf16 (halves store DMA). Host
    upcasts to fp32 and adds the bias during the unshard (exact in fp32).
    Measured rel-l2 vs fp32 reference: ~2.6e-3, far under the 2e-2 gate.
  * Whole x (6.3 MB bf16) and W (3.5 MB bf16) are SBUF-resident; x streams
    on the sync (SP) HWDGE queue while W pieces + y stores share the
    scalar (ACT) queue, so weight prefetch never queues behind stores.
  * DMA completion semaphores: a DMA's +16 arrives as 16 independent
    per-SDMA-engine +1s, so cumulative waits on a shared semaphore are
    UNSOUND with >1 DMA of that semaphore in flight (engines race ahead).
    Each DMA stream uses K=3 rotating semaphores with a producer-side
    throttle (issue of DMA j waits for DMA j-K's completion), so each
    semaphore has at most one in-flight incrementer and full-count waits
    are exact. Store completions rotate 2 sems by supertile parity (the
    next same-parity store group is data-dependent on the wait, so no
    throttle needed).
  * Supertile loop is chunk-column-major so the first m-tiles only need
    the first 512 W columns; first x supertile and the first W column
    chunk are split fine so the PE starts ~3 us in.
  * Raw-bass pipeline (explicit semaphores); PSUM->SBUF eviction (with the
    fp32->bf16 cast) on the DVE rotates through all 8 PSUM banks.
  * The first start-of-group N=512 matmul is emitted twice: this walrus
    build eats one early MATMUL (verified via the NTFF instruction
    stream); the duplicate is idempotent (start=True clears the bank).
"""

from contextlib import ExitStack

import ml_dtypes
import numpy as np

import concourse.bass as bass
import concourse.mybir as mybir
from concourse.bass_utils import run_bass_kernel_spmd

P = 128
DIM = 768
NOUT = 3 * DIM          # 2304
KT = DIM // P           # 6 k-tiles
B = 8                   # batch == n_cores
M = 64 * 64             # 4096 tokens per core
MT = M // P             # 32 m-tiles per core
TG = 512                # token supertile
NG = M // TG            # 8 supertiles
MT_G = TG // P          # 4 m-tiles per supertile
N_CHUNKS = [(0, 512), (512, 512), (1024, 512), (1536, 512), (2048, 256)]
NCH = len(N_CHUNKS)     # 5 chunk columns
N_PSUM = 8              # all psum banks
OB = 3                  # output staging buffers (supertile granularity)
KSEM = 3                # rotating DMA-completion sems per input stream

_F32 = mybir.dt.float32
_BF16 = mybir.dt.bfloat16


def _build_program():
    nc = bass.Bass()
    xt = nc.dram_tensor("xt", [P, MT, KT, P], _BF16, kind="ExternalInput")
    # piece-major W: per partition, concat over chunk-columns c of the
    # [KT, nsz_c] k-major block -> every W DMA is one contiguous run
    wt = nc.dram_tensor("wt", [P, KT * NOUT], _BF16, kind="ExternalInput")
    y = nc.dram_tensor("y", [M, NOUT], _BF16, kind="ExternalOutput")

    with ExitStack() as ctx:
        x_sb = ctx.enter_context(nc.sbuf_tensor("x_sb", [P, MT, KT, P], _BF16))
        wt_sb = ctx.enter_context(nc.sbuf_tensor("wt_sb", [P, KT, NOUT], _BF16))
        o_sb = ctx.enter_context(nc.sbuf_tensor("o_sb", [P, OB, MT_G, NOUT], _BF16))
        ps = [
            ctx.enter_context(nc.psum_tensor(f"ps{i}", [P, 512], _F32))
            for i in range(N_PSUM)
        ]
        sx = [ctx.enter_context(nc.semaphore(f"sx{i}")) for i in range(KSEM)]
        sw = [ctx.enter_context(nc.semaphore(f"sw{i}")) for i in range(KSEM)]
        so = [ctx.enter_context(nc.semaphore(f"so{i}")) for i in range(OB)]
        s_mm = ctx.enter_context(nc.semaphore("s_mm"))
        s_tt = ctx.enter_context(nc.semaphore("s_tt"))
        block = ctx.enter_context(nc.Block())

        # x milestones: j=0..3 granules of supertile 0, j=4..10 supertiles 1..7
        # w milestones: j=0..5 k-granules of chunk-col 0, j=6..9 pieces c=1..4
        def x_done(j):
            return sx[j % KSEM], 16 * (j // KSEM + 1)

        def w_done(j):
            return sw[j % KSEM], 16 * (j // KSEM + 1)

        @block.sync
        def _(sync):
            for j in range(MT_G + NG - 1):
                if j >= KSEM:
                    sem, val = x_done(j - KSEM)
                    sync.wait_ge(sem, val)
                if j < MT_G:
                    d = sync.dma_start(out=x_sb[:, j], in_=xt[:, j])
                else:
                    g = j - MT_G + 1
                    if g >= 2:
                        # Just-in-time: don't fight the W pieces for HBM
                        # bandwidth during the ramp. Supertile g is needed
                        # one full supertile (~26 us) after this fires.
                        sync.wait_ge(s_tt, NCH * MT_G * (g - 2) + 1)
                    d = sync.dma_start(
                        out=x_sb[:, MT_G * g : MT_G * (g + 1)],
                        in_=xt[:, MT_G * g : MT_G * (g + 1)],
                    )
                d.then_inc(sx[j % KSEM], 16)

        @block.scalar
        def _(scalar):
            nszc0 = N_CHUNKS[0][1]
            for j in range(KT + NCH - 1):
                if j >= KSEM:
                    sem, val = w_done(j - KSEM)
                    scalar.wait_ge(sem, val)
                if j < KT:
                    # k-granule j of chunk-column 0
                    d = scalar.dma_start(
                        out=wt_sb[:, j, 0:nszc0],
                        in_=wt[:, j * nszc0 : (j + 1) * nszc0],
                    )
                else:
                    n0, nsz = N_CHUNKS[j - KT + 1]
                    d = scalar.dma_start(
                        out=wt_sb[:, :, n0 : n0 + nsz],
                        in_=wt[:, KT * n0 : KT * (n0 + nsz)],
                    )
                d.then_inc(sw[j % KSEM], 16)
            # y stores (behind the W pieces on the same FIFO queue).
            for g in range(NG):
                for ms in range(MT_G):
                    ma = MT_G * g + ms
                    # all NCH chunk-columns of this m-tile evicted
                    if g == NG - 1:
                        scalar.wait_ge(s_tt, NCH * MT_G * g + NCH * (ms + 1))
                    else:
                        scalar.wait_ge(
                            s_tt, NCH * MT_G * g + (NCH - 1) * MT_G + ms + 1
                        )
                    scalar.dma_start(
                        out=y[ma * P : (ma + 1) * P, :], in_=o_sb[:, g % OB, ms, :]
                    ).then_inc(so[g % OB], 16)

        def chunk_order(g):
            # chunk-column-major while W streams in; m-tile-major on the
            # last supertile so its stores overlap the remaining compute
            if g == NG - 1:
                return [(c, ms) for ms in range(MT_G) for c in range(NCH)]
            return [(c, ms) for c in range(NCH) for ms in range(MT_G)]

        @block.tensor
        def _(tensor):
            # HAM warmup: keep the PE busy during the initial DMA ramp so
            # the clock gate is at 8/8 when the real stream starts. Junk
            # inputs; output bank is cleared later by a start=True group.
            for _ in range(18):
                nc.tensor.matmul(
                    ps[N_PSUM - 1][:, :128],
                    lhsT=x_sb[:, 0, 0, :],
                    rhs=wt_sb[:, 0, 0:128],
                    start=True,
                    stop=True,
                    skip_group_check=True,
                )
            cyc = 0
            for g in range(NG):
                for c, ms in chunk_order(g):
                    n0, nsz = N_CHUNKS[c]
                    if True:
                        ma = MT_G * g + ms
                        if g == 0:
                            if c == 0:
                                sem, val = x_done(ms)
                                tensor.wait_ge(sem, val)
                            elif ms == 0:
                                sem, val = w_done(KT + c - 1)
                                tensor.wait_ge(sem, val)
                        elif c == 0 and ms == 0:
                            sem, val = x_done(MT_G + g - 1)
                            tensor.wait_ge(sem, val)
                        if cyc >= N_PSUM:
                            # DVE finished reading this psum bank
                            tensor.wait_ge(s_tt, cyc - N_PSUM + 1)
                        for k in range(KT):
                            if g == 0 and c == 0 and ms == 0:
                                sem, val = w_done(k)
                                tensor.wait_ge(sem, val)
                                if k == 0:
                                    # Sacrificial duplicate of the k=0
                                    # matmul: this walrus build eats the
                                    # first start-of-group N=512 MATMUL
                                    # (observed in the NTFF stream across
                                    # three variants). start=True clears
                                    # the bank, so whichever copy survives
                                    # the result is correct.
                                    nc.tensor.matmul(
                                        ps[0][:, :nsz],
                                        lhsT=x_sb[:, 0, 0, :],
                                        rhs=wt_sb[:, 0, n0 : n0 + nsz],
                                        start=True,
                                        stop=False,
                                        skip_group_check=True,
                                    )
                            mm = nc.tensor.matmul(
                                ps[cyc % N_PSUM][:, :nsz],
                                lhsT=x_sb[:, ma, k, :],
                                rhs=wt_sb[:, k, n0 : n0 + nsz],
                                start=(k == 0),
                                stop=(k == KT - 1),
                            )
                        mm.then_inc(s_mm, 1)
                        cyc += 1

        @block.vector
        def _(vector):
            cyc = 0
            for g in range(NG):
                for i, (c, ms) in enumerate(chunk_order(g)):
                    n0, nsz = N_CHUNKS[c]
                    vector.wait_ge(s_mm, cyc + 1)
                    if i == 0 and g >= OB:
                        # staging buffer free once supertile g-OB stored
                        vector.wait_ge(so[g % OB], 16 * MT_G * (g // OB))
                    nc.vector.tensor_copy(
                        o_sb[:, g % OB, ms, n0 : n0 + nsz],
                        ps[cyc % N_PSUM][:, :nsz],
                    ).then_inc(s_tt, 1)
                    cyc += 1

    return nc


def _prepare_inputs(x, Wqkv, bqkv, Aq, Bq, Ak, Bk, Av, Bv):
    x = np.asarray(x, dtype=np.float32)
    Wqkv = np.asarray(Wqkv, dtype=np.float32)

    # Fold LoRA: W_eff[j-th slice] = Wqkv[j-th slice] + B_j @ A_j
    w_eff = Wqkv.copy()
    for j, (A, Bm) in enumerate(((Aq, Bq), (Ak, Bk), (Av, Bv))):
        A = np.asarray(A, dtype=np.float32)
        Bm = np.asarray(Bm, dtype=np.float32)
        w_eff[j * DIM : (j + 1) * DIM] += Bm @ A

    # wt3[p, k, n] = W_eff[n, k*128+p]; DRAM is piece-major: per partition,
    # concat over chunk-columns c of the k-major [KT, nsz_c] block, so every
    # W DMA reads one contiguous run per partition.
    wt3 = w_eff.reshape(NOUT, KT, P).transpose(2, 1, 0)
    wt = np.ascontiguousarray(
        np.concatenate(
            [wt3[:, :, n0 : n0 + nsz].reshape(P, KT * nsz) for n0, nsz in N_CHUNKS],
            axis=1,
        ).astype(ml_dtypes.bfloat16)
    )

    in_maps = []
    for b in range(B):
        # xt[p, ma, k, t] = x[b, ma*128+t, k*128+p], bf16
        xb = x[b].reshape(MT, P, KT, P)
        xtb = np.ascontiguousarray(
            xb.transpose(3, 0, 2, 1).astype(ml_dtypes.bfloat16)
        )
        in_maps.append({"xt": xtb, "wt": wt})
    return in_maps


def _run(inputs, trace=False, trace_kwargs=None):
    nc = _build_program()
    in_maps = _prepare_inputs(**inputs)
    res = run_bass_kernel_spmd(
        nc,
        in_maps,
        core_ids=list(range(B)),
        trace=trace,
        **(trace_kwargs or {}),
    )
    bqkv = np.asarray(inputs["bqkv"], dtype=np.float32)
    outs = res.results
    y = np.stack(
        [
            np.asarray(outs[b]["y"]).astype(np.float32).reshape(64, 64, NOUT) + bqkv
            for b in range(B)
        ]
    )
    return y, res


def kernel(**inputs):
    y, _ = _run(inputs, trace=False)
    return y


# revision 19
# speedup vs baseline: 1.0159x; 1.0159x over previous
"""LoRA QKV projection kernel for 8 Trainium2 NeuronCores.

Reference computation (per problem):
    qkv = x @ Wqkv^T + bqkv + concat(x@Aq^T@Bq^T, x@Ak^T@Bk^T, x@Av^T@Bv^T)

Strategy:
  * Host folds the rank-16 LoRA factors into the dense weight
    (W_eff = Wqkv + blockdiag(BqAq, BkAk, BvAv) — ~56 MFLOP, 0.05% of the
    116 GFLOP GEMM), so the device runs one pure GEMM at the roofline ridge.
  * Data-parallel: batch dim (8) sharded 1:1 over the 8 cores.
    Each core: y[4096, 2304] = x_b[4096, 768] @ W_eff^T.
  * bf16 everywhere off-chip: x and W_eff are cast to bf16 on host (halves
    input DMA, enables fast-weight-load on the PE), the GEMM accumulates in
    fp32 PSUM, and the output is stored as bf16 (halves store DMA). Host
    upcasts to fp32 and adds the bias during the unshard (exact in fp32).
    Measured rel-l2 vs fp32 reference: ~2.6e-3, far under the 2e-2 gate.
  * Whole x (6.3 MB bf16) and W (3.5 MB bf16) are SBUF-resident; x streams
    on the sync (SP) HWDGE queue while W pieces + y stores share the
    scalar (ACT) queue, so weight prefetch never queues behind stores.
  * DMA completion semaphores: a DMA's +16 arrives as 16 independent
    per-SDMA-engine +1s, so cumulative waits on a shared semaphore are
    UNSOUND with >1 DMA of that semaphore in flight (engines race ahead).
    Each DMA stream uses K=3 rotating semaphores with a producer-side
    throttle (issue of DMA j waits for DMA j-K's completion), so each
    semaphore has at most one in-flight incrementer and full-count waits
    are exact. Store completions rotate 2 sems by supertile parity (the
    next same-parity store group is data-dependent on the wait, so no
    throttle needed).
  * Supertile loop is chunk-column-major so the first m-tiles only need
    the first 512 W columns; first x supertile and the first W column
    chunk are split fine so the PE starts ~3 us in.
  * Raw-bass pipeline (explicit semaphores); PSUM->SBUF eviction (with the
    fp32->bf16 cast) on the DVE rotates through all 8 PSUM banks.
  * The first start-of-group N=512 matmul is emitted twice: this walrus
    build eats one early MATMUL (verified via the NTFF instruction
    stream); the duplicate is idempotent (start=True clears the bank).
"""

from contextlib import ExitStack

import ml_dtypes
import numpy as np

import concourse.bass as bass
import concourse.mybir as mybir
from concourse.bass_utils import run_bass_kernel_spmd

P = 128
DIM = 768
NOUT = 3 * DIM          # 2304
KT = DIM // P           # 6 k-tiles
B = 8                   # batch == n_cores
M = 64 * 64             # 4096 tokens per core
MT = M // P             # 32 m-tiles per core
TG = 512                # token supertile
NG = M // TG            # 8 supertiles
MT_G = TG // P          # 4 m-tiles per supertile
N_CHUNKS = [(0, 512), (512, 512), (1024, 512), (1536, 512), (2048, 256)]
NCH = len(N_CHUNKS)     # 5 chunk columns
W_GROUPS = [(0, 512), (512, 512), (1024, 1280)]  # W DMA granularity
N_PSUM = 8              # all psum banks
OB = 3                  # output staging buffers (supertile granularity)
KSEM = 3                # rotating DMA-completion sems per input stream

_F32 = mybir.dt.float32
_BF16 = mybir.dt.bfloat16


def _build_program():
    nc = bass.Bass()
    xt = nc.dram_tensor("xt", [P, MT, KT, P], _BF16, kind="ExternalInput")
    # piece-major W: per partition, concat over chunk-columns c of the
    # [KT, nsz_c] k-major block -> every W DMA is one contiguous run
    wt = nc.dram_tensor("wt", [P, KT * NOUT], _BF16, kind="ExternalInput")
    y = nc.dram_tensor("y", [M, NOUT], _BF16, kind="ExternalOutput")

    with ExitStack() as ctx:
        x_sb = ctx.enter_context(nc.sbuf_tensor("x_sb", [P, MT, KT, P], _BF16))
        wt_sb = ctx.enter_context(nc.sbuf_tensor("wt_sb", [P, KT, NOUT], _BF16))
        o_sb = ctx.enter_context(nc.sbuf_tensor("o_sb", [P, OB, MT_G, NOUT], _BF16))
        ps = [
            ctx.enter_context(nc.psum_tensor(f"ps{i}", [P, 512], _F32))
            for i in range(N_PSUM)
        ]
        sx = [ctx.enter_context(nc.semaphore(f"sx{i}")) for i in range(KSEM)]
        sw = [ctx.enter_context(nc.semaphore(f"sw{i}")) for i in range(KSEM)]
        so = [ctx.enter_context(nc.semaphore(f"so{i}")) for i in range(OB)]
        s_mm = ctx.enter_context(nc.semaphore("s_mm"))
        s_tt = ctx.enter_context(nc.semaphore("s_tt"))
        block = ctx.enter_context(nc.Block())

        # Each HWDGE DMA carries ~2-3 us of ring/receipt overhead regardless
        # of size, so both input streams use few, large transfers.
        # x milestones: j = supertile g (8 DMAs). w milestones: 3 DMAs —
        # chunk-col 0, chunk-col 1, chunk-cols 2..4.
        def x_done(j):
            return sx[j % KSEM], 16 * (j // KSEM + 1)

        def w_done(j):
            return sw[j], 16

        @block.sync
        def _(sync):
            for g in range(NG):
                if g >= KSEM:
                    sem, val = x_done(g - KSEM)
                    sync.wait_ge(sem, val)
                if g >= 2:
                    # Just-in-time: don't fight the W pieces for HBM
                    # bandwidth during the ramp. Supertile g is needed
                    # one full supertile (~26 us) after this fires.
                    sync.wait_ge(s_tt, NCH * MT_G * (g - 2) + 1)
                sync.dma_start(
                    out=x_sb[:, MT_G * g : MT_G * (g + 1)],
                    in_=xt[:, MT_G * g : MT_G * (g + 1)],
                ).then_inc(sx[g % KSEM], 16)

        @block.scalar
        def _(scalar):
            for j, (n0, nsz) in enumerate(W_GROUPS):
                scalar.dma_start(
                    out=wt_sb[:, :, n0 : n0 + nsz],
                    in_=wt[:, KT * n0 : KT * (n0 + nsz)],
                ).then_inc(sw[j], 16)
            # y stores (behind the W pieces on the same FIFO queue).
            for g in range(NG):
                for ms in range(MT_G):
                    ma = MT_G * g + ms
                    # all NCH chunk-columns of this m-tile evicted
                    if g == NG - 1:
                        scalar.wait_ge(s_tt, NCH * MT_G * g + NCH * (ms + 1))
                    else:
                        scalar.wait_ge(
                            s_tt, NCH * MT_G * g + (NCH - 1) * MT_G + ms + 1
                        )
                    scalar.dma_start(
                        out=y[ma * P : (ma + 1) * P, :], in_=o_sb[:, g % OB, ms, :]
                    ).then_inc(so[g % OB], 16)

        def chunk_order(g):
            # chunk-column-major while W streams in; m-tile-major on the
            # last supertile so its stores overlap the remaining compute
            if g == NG - 1:
                return [(c, ms) for ms in range(MT_G) for c in range(NCH)]
            return [(c, ms) for c in range(NCH) for ms in range(MT_G)]

        @block.tensor
        def _(tensor):
            # HAM warmup: keep the PE busy during the initial DMA ramp so
            # the clock gate is at 8/8 when the real stream starts. Junk
            # inputs; output bank is cleared later by a start=True group.
            for _ in range(18):
                nc.tensor.matmul(
                    ps[N_PSUM - 1][:, :128],
                    lhsT=x_sb[:, 0, 0, :],
                    rhs=wt_sb[:, 0, 0:128],
                    start=True,
                    stop=True,
                    skip_group_check=True,
                )
            cyc = 0
            for g in range(NG):
                for c, ms in chunk_order(g):
                    n0, nsz = N_CHUNKS[c]
                    if True:
                        ma = MT_G * g + ms
                        if cyc % (NCH * MT_G) == 0:
                            # supertile g's x loaded
                            sem, val = x_done(g)
                            tensor.wait_ge(sem, val)
                        if g == 0 and ms == 0 and c < len(W_GROUPS):
                            # W group c loaded
                            sem, val = w_done(c)
                            tensor.wait_ge(sem, val)
                        if cyc >= N_PSUM:
                            # DVE finished reading this psum bank
                            tensor.wait_ge(s_tt, cyc - N_PSUM + 1)
                        for k in range(KT):
                            if g == 0 and c == 0 and ms == 0:
                                if k == 0:
                                    # Sacrificial duplicate of the k=0
                                    # matmul: this walrus build eats the
                                    # first start-of-group N=512 MATMUL
                                    # (observed in the NTFF stream across
                                    # three variants). start=True clears
                                    # the bank, so whichever copy survives
                                    # the result is correct.
                                    nc.tensor.matmul(
                                        ps[0][:, :nsz],
                                        lhsT=x_sb[:, 0, 0, :],
                                        rhs=wt_sb[:, 0, n0 : n0 + nsz],
                                        start=True,
                                        stop=False,
                                        skip_group_check=True,
                                    )
                            mm = nc.tensor.matmul(
                                ps[cyc % N_PSUM][:, :nsz],
                                lhsT=x_sb[:, ma, k, :],
                                rhs=wt_sb[:, k, n0 : n0 + nsz],
                                start=(k == 0),
                                stop=(k == KT - 1),
                            )
                        mm.then_inc(s_mm, 1)
                        cyc += 1

        @block.vector
        def _(vector):
            cyc = 0
            for g in range(NG):
                for i, (c, ms) in enumerate(chunk_order(g)):
                    n0, nsz = N_CHUNKS[c]
                    vector.wait_ge(s_mm, cyc + 1)
                    if i == 0 and g >= OB:
                        # staging buffer free once supertile g-OB stored
                        vector.wait_ge(so[g % OB], 16 * MT_G * (g // OB))
                    nc.vector.tensor_copy(
                        o_sb[:, g % OB, ms, n0 : n0 + nsz],
                        ps[cyc % N_PSUM][:, :nsz],
                    ).then_inc(s_tt, 1)
                    cyc += 1

    return nc


def _prepare_inputs(x, Wqkv, bqkv, Aq, Bq, Ak, Bk, Av, Bv):
    x = np.asarray(x, dtype=np.float32)
    Wqkv = np.asarray(Wqkv, dtype=np.float32)

    # Fold LoRA: W_eff[j-th slice] = Wqkv[j-th slice] + B_j @ A_j
    w_eff = Wqkv.copy()
    for j, (A, Bm) in enumerate(((Aq, Bq), (Ak, Bk), (Av, Bv))):
        A = np.asarray(A, dtype=np.float32)
        Bm = np.asarray(Bm, dtype=np.float32)
        w_eff[j * DIM : (j + 1) * DIM] += Bm @ A

    # wt3[p, k, n] = W_eff[n, k*128+p]; DRAM is group-major: per partition,
    # concat over the W DMA groups of the k-major [KT, nsz_g] block, so every
    # W DMA reads one contiguous run per partition in SBUF iteration order.
    wt3 = w_eff.reshape(NOUT, KT, P).transpose(2, 1, 0)
    wt = np.ascontiguousarray(
        np.concatenate(
            [wt3[:, :, n0 : n0 + nsz].reshape(P, KT * nsz) for n0, nsz in W_GROUPS],
            axis=1,
        ).astype(ml_dtypes.bfloat16)
    )

    in_maps = []
    for b in range(B):
        # xt[p, ma, k, t] = x[b, ma*128+t, k*128+p], bf16
        xb = x[b].reshape(MT, P, KT, P)
        xtb = np.ascontiguousarray(
            xb.transpose(3, 0, 2, 1).astype(ml_dtypes.bfloat16)
        )
        in_maps.append({"xt": xtb, "wt": wt})
    return in_maps


def _run(inputs, trace=False, trace_kwargs=None):
    nc = _build_program()
    in_maps = _prepare_inputs(**inputs)
    res = run_bass_kernel_spmd(
        nc,
        in_maps,
        core_ids=list(range(B)),
        trace=trace,
        **(trace_kwargs or {}),
    )
    bqkv = np.asarray(inputs["bqkv"], dtype=np.float32)
    outs = res.results
    y = np.stack(
        [
            np.asarray(outs[b]["y"]).astype(np.float32).reshape(64, 64, NOUT) + bqkv
            for b in range(B)
        ]
    )
    return y, res


def kernel(**inputs):
    y, _ = _run(inputs, trace=False)
    return y


# revision 24
# speedup vs baseline: 1.0374x; 1.0212x over previous
"""LoRA QKV projection kernel for 8 Trainium2 NeuronCores.

Reference computation (per problem):
    qkv = x @ Wqkv^T + bqkv + concat(x@Aq^T@Bq^T, x@Ak^T@Bk^T, x@Av^T@Bv^T)

Strategy:
  * Host folds the rank-16 LoRA factors into the dense weight
    (W_eff = Wqkv + blockdiag(BqAq, BkAk, BvAv) — ~56 MFLOP, 0.05% of the
    116 GFLOP GEMM), so the device runs one pure GEMM at the roofline ridge.
  * Data-parallel: batch dim (8) sharded 1:1 over the 8 cores.
    Each core: y[4096, 2304] = x_b[4096, 768] @ W_eff^T.
  * bf16 everywhere off-chip: x and W_eff are cast to bf16 on host (halves
    input DMA, enables fast-weight-load on the PE), the GEMM accumulates in
    fp32 PSUM, and the output is stored as bf16 (halves store DMA). Host
    upcasts to fp32 and adds the bias during the unshard (exact in fp32).
    Measured rel-l2 vs fp32 reference: ~2.6e-3, far under the 2e-2 gate.
  * Whole x (6.3 MB bf16) and W (3.5 MB bf16) are SBUF-resident; x streams
    on the sync (SP) HWDGE queue while W pieces + y stores share the
    scalar (ACT) queue, so weight prefetch never queues behind stores.
  * DMA completion semaphores: a DMA's +16 arrives as 16 independent
    per-SDMA-engine +1s, so cumulative waits on a shared semaphore are
    UNSOUND with >1 DMA of that semaphore in flight (engines race ahead).
    Each DMA stream uses K=3 rotating semaphores with a producer-side
    throttle (issue of DMA j waits for DMA j-K's completion), so each
    semaphore has at most one in-flight incrementer and full-count waits
    are exact. Store completions rotate 2 sems by supertile parity (the
    next same-parity store group is data-dependent on the wait, so no
    throttle needed).
  * Supertile loop is chunk-column-major so the first m-tiles only need
    the first 512 W columns; first x supertile and the first W column
    chunk are split fine so the PE starts ~3 us in.
  * Raw-bass pipeline (explicit semaphores); PSUM->SBUF eviction (with the
    fp32->bf16 cast) on the DVE rotates through all 8 PSUM banks.
  * The first start-of-group N=512 matmul is emitted twice: this walrus
    build eats one early MATMUL (verified via the NTFF instruction
    stream); the duplicate is idempotent (start=True clears the bank).
"""

from contextlib import ExitStack

import ml_dtypes
import numpy as np

import concourse.bass as bass
import concourse.mybir as mybir
from concourse.bass_utils import run_bass_kernel_spmd

P = 128
DIM = 768
NOUT = 3 * DIM          # 2304
KT = DIM // P           # 6 k-tiles
B = 8                   # batch == n_cores
M = 64 * 64             # 4096 tokens per core
MT = M // P             # 32 m-tiles per core
TG = 512                # token supertile
NG = M // TG            # 8 supertiles
MT_G = TG // P          # 4 m-tiles per supertile

# Chunk-column processing order: the narrow 256-col chunk first, so the
# first W DMA (one group per leading chunk) is small and compute starts
# earlier. W_GROUPS must cover N_CHUNKS prefixes in this order.
N_CHUNKS = [(2048, 256), (0, 512), (512, 512), (1024, 512), (1536, 512)]
NCH = len(N_CHUNKS)     # 5 chunk columns
W_GROUPS = [(2048, 256), (0, 512), (512, 512), (1024, 1024)]  # W DMA granularity
# chunk position i -> W group that must be resident (None = already covered)
W_WAIT_OF_CHUNK = [0, 1, 2, 3, None]
N_PSUM = 8              # all psum banks
OB = 3                  # output staging buffers (supertile granularity)
KSEM = 3                # rotating DMA-completion sems per input stream

_F32 = mybir.dt.float32
_BF16 = mybir.dt.bfloat16


def _build_program():
    nc = bass.Bass()
    xt = nc.dram_tensor("xt", [P, MT, KT, P], _BF16, kind="ExternalInput")
    # piece-major W: per partition, concat over chunk-columns c of the
    # [KT, nsz_c] k-major block -> every W DMA is one contiguous run
    wt = nc.dram_tensor("wt", [P, KT * NOUT], _BF16, kind="ExternalInput")
    y = nc.dram_tensor("y", [M, NOUT], _BF16, kind="ExternalOutput")

    with ExitStack() as ctx:
        x_sb = ctx.enter_context(nc.sbuf_tensor("x_sb", [P, MT, KT, P], _BF16))
        wt_sb = ctx.enter_context(nc.sbuf_tensor("wt_sb", [P, KT, NOUT], _BF16))
        o_sb = ctx.enter_context(nc.sbuf_tensor("o_sb", [P, OB, MT_G, NOUT], _BF16))
        ps = [
            ctx.enter_context(nc.psum_tensor(f"ps{i}", [P, 512], _F32))
            for i in range(N_PSUM)
        ]
        sx = [ctx.enter_context(nc.semaphore(f"sx{i}")) for i in range(KSEM)]
        sw = [ctx.enter_context(nc.semaphore(f"sw{i}")) for i in range(len(W_GROUPS))]
        so = [ctx.enter_context(nc.semaphore(f"so{i}")) for i in range(OB)]
        s_mm = ctx.enter_context(nc.semaphore("s_mm"))
        s_tt = ctx.enter_context(nc.semaphore("s_tt"))
        block = ctx.enter_context(nc.Block())

        # Each HWDGE DMA carries ~2-3 us of ring/receipt overhead regardless
        # of size, so both input streams use few, large transfers.
        # x milestones: j = supertile g (8 DMAs). w milestones: 3 DMAs —
        # chunk-col 0, chunk-col 1, chunk-cols 2..4.
        def x_done(j):
            return sx[j % KSEM], 16 * (j // KSEM + 1)

        def w_done(j):
            return sw[j], 16

        @block.sync
        def _(sync):
            # j=0,1: halves of supertile 0 (small first transfer -> early
            # compute start); j=g+1: whole supertile g for g>=1.
            for j in range(NG + 1):
                if j >= KSEM:
                    sem, val = x_done(j - KSEM)
                    sync.wait_ge(sem, val)
                if j < 2:
                    lo, hi = 2 * j, 2 * (j + 1)
                else:
                    g = j - 1
                    # Just-in-time: don't fight the W pieces for HBM
                    # bandwidth during the ramp. Supertile g is needed
                    # one full supertile (~26 us) after this fires.
                    sync.wait_ge(s_tt, max(NCH * MT_G * (g - 2), 0) + 1)
                    lo, hi = MT_G * g, MT_G * (g + 1)
                sync.dma_start(out=x_sb[:, lo:hi], in_=xt[:, lo:hi]).then_inc(
                    sx[j % KSEM], 16
                )

        @block.scalar
        def _(scalar):
            off = 0
            for j, (n0, nsz) in enumerate(W_GROUPS):
                scalar.dma_start(
                    out=wt_sb[:, :, n0 : n0 + nsz],
                    in_=wt[:, off : off + KT * nsz],
                ).then_inc(sw[j], 16)
                off += KT * nsz
            # y stores (behind the W pieces on the same FIFO queue).
            for g in range(NG):
                for ms in range(MT_G):
                    ma = MT_G * g + ms
                    # all NCH chunk-columns of this m-tile evicted
                    if g == NG - 1:
                        scalar.wait_ge(s_tt, NCH * MT_G * g + NCH * (ms + 1))
                    else:
                        scalar.wait_ge(
                            s_tt, NCH * MT_G * g + (NCH - 1) * MT_G + ms + 1
                        )
                    scalar.dma_start(
                        out=y[ma * P : (ma + 1) * P, :], in_=o_sb[:, g % OB, ms, :]
                    ).then_inc(so[g % OB], 16)

        def chunk_order(g):
            # chunk-column-major while W streams in; m-tile-major on the
            # last supertile so its stores overlap the remaining compute
            if g == NG - 1:
                return [(c, ms) for ms in range(MT_G) for c in range(NCH)]
            return [(c, ms) for c in range(NCH) for ms in range(MT_G)]

        @block.tensor
        def _(tensor):
            # HAM warmup: keep the PE busy during the initial DMA ramp so
            # the clock gate is at 8/8 when the real stream starts. Junk
            # inputs; output bank is cleared later by a start=True group.
            for _ in range(18):
                nc.tensor.matmul(
                    ps[N_PSUM - 1][:, :128],
                    lhsT=x_sb[:, 0, 0, :],
                    rhs=wt_sb[:, 0, 0:128],
                    start=True,
                    stop=True,
                    skip_group_check=True,
                )
            cyc = 0
            for g in range(NG):
                for c, ms in chunk_order(g):
                    n0, nsz = N_CHUNKS[c]
                    if True:
                        ma = MT_G * g + ms
                        if g == 0:
                            if c == 0:
                                # half-supertile ms//2 of x loaded
                                sem, val = x_done(ms // 2)
                                tensor.wait_ge(sem, val)
                            if ms == 0 and W_WAIT_OF_CHUNK[c] is not None:
                                sem, val = w_done(W_WAIT_OF_CHUNK[c])
                                tensor.wait_ge(sem, val)
                        elif cyc % (NCH * MT_G) == 0:
                            # supertile g's x loaded (milestone j = g+1)
                            sem, val = x_done(g + 1)
                            tensor.wait_ge(sem, val)
                        if cyc >= N_PSUM:
                            # DVE finished reading this psum bank
                            tensor.wait_ge(s_tt, cyc - N_PSUM + 1)
                        for k in range(KT):
                            if g == 0 and c <= 1 and ms == 0 and k == 0:
                                # Sacrificial duplicate of the k=0 matmul:
                                # this walrus build eats one early
                                # start-of-group MATMUL (observed in the
                                # NTFF stream across variants). start=True
                                # clears the bank, so whichever copy
                                # survives the result is correct. Emitted
                                # for the first two chunk-columns to cover
                                # both the N=256 and N=512 leading groups.
                                nc.tensor.matmul(
                                    ps[cyc % N_PSUM][:, :nsz],
                                    lhsT=x_sb[:, 0, 0, :],
                                    rhs=wt_sb[:, 0, n0 : n0 + nsz],
                                    start=True,
                                    stop=False,
                                    skip_group_check=True,
                                )
                            mm = nc.tensor.matmul(
                                ps[cyc % N_PSUM][:, :nsz],
                                lhsT=x_sb[:, ma, k, :],
                                rhs=wt_sb[:, k, n0 : n0 + nsz],
                                start=(k == 0),
                                stop=(k == KT - 1),
                            )
                        mm.then_inc(s_mm, 1)
                        cyc += 1

        @block.vector
        def _(vector):
            cyc = 0
            for g in range(NG):
                for i, (c, ms) in enumerate(chunk_order(g)):
                    n0, nsz = N_CHUNKS[c]
                    vector.wait_ge(s_mm, cyc + 1)
                    if i == 0 and g >= OB:
                        # staging buffer free once supertile g-OB stored
                        vector.wait_ge(so[g % OB], 16 * MT_G * (g // OB))
                    nc.vector.tensor_copy(
                        o_sb[:, g % OB, ms, n0 : n0 + nsz],
                        ps[cyc % N_PSUM][:, :nsz],
                    ).then_inc(s_tt, 1)
                    cyc += 1

    return nc


def _prepare_inputs(x, Wqkv, bqkv, Aq, Bq, Ak, Bk, Av, Bv):
    x = np.asarray(x, dtype=np.float32)
    Wqkv = np.asarray(Wqkv, dtype=np.float32)

    # Fold LoRA: W_eff[j-th slice] = Wqkv[j-th slice] + B_j @ A_j
    w_eff = Wqkv.copy()
    for j, (A, Bm) in enumerate(((Aq, Bq), (Ak, Bk), (Av, Bv))):
        A = np.asarray(A, dtype=np.float32)
        Bm = np.asarray(Bm, dtype=np.float32)
        w_eff[j * DIM : (j + 1) * DIM] += Bm @ A

    # wt3[p, k, n] = W_eff[n, k*128+p]; DRAM is group-major: per partition,
    # concat over the W DMA groups of the k-major [KT, nsz_g] block, so every
    # W DMA reads one contiguous run per partition in SBUF iteration order.
    wt3 = w_eff.reshape(NOUT, KT, P).transpose(2, 1, 0)
    wt = np.ascontiguousarray(
        np.concatenate(
            [wt3[:, :, n0 : n0 + nsz].reshape(P, KT * nsz) for n0, nsz in W_GROUPS],
            axis=1,
        ).astype(ml_dtypes.bfloat16)
    )

    in_maps = []
    for b in range(B):
        # xt[p, ma, k, t] = x[b, ma*128+t, k*128+p], bf16
        xb = x[b].reshape(MT, P, KT, P)
        xtb = np.ascontiguousarray(
            xb.transpose(3, 0, 2, 1).astype(ml_dtypes.bfloat16)
        )
        in_maps.append({"xt": xtb, "wt": wt})
    return in_maps


def _run(inputs, trace=False, trace_kwargs=None):
    nc = _build_program()
    in_maps = _prepare_inputs(**inputs)
    res = run_bass_kernel_spmd(
        nc,
        in_maps,
        core_ids=list(range(B)),
        trace=trace,
        **(trace_kwargs or {}),
    )
    bqkv = np.asarray(inputs["bqkv"], dtype=np.float32)
    outs = res.results
    y = np.stack(
        [
            np.asarray(outs[b]["y"]).astype(np.float32).reshape(64, 64, NOUT) + bqkv
            for b in range(B)
        ]
    )
    return y, res


def kernel(**inputs):
    y, _ = _run(inputs, trace=False)
    return y


# revision 25
# speedup vs baseline: 1.0382x; 1.0007x over previous
"""LoRA QKV projection kernel for 8 Trainium2 NeuronCores.

Reference computation (per problem):
    qkv = x @ Wqkv^T + bqkv + concat(x@Aq^T@Bq^T, x@Ak^T@Bk^T, x@Av^T@Bv^T)

Strategy:
  * Host folds the rank-16 LoRA factors into the dense weight
    (W_eff = Wqkv + blockdiag(BqAq, BkAk, BvAv) — ~56 MFLOP, 0.05% of the
    116 GFLOP GEMM), so the device runs one pure GEMM at the roofline ridge.
  * Data-parallel: batch dim (8) sharded 1:1 over the 8 cores.
    Each core: y[4096, 2304] = x_b[4096, 768] @ W_eff^T.
  * bf16 everywhere off-chip: x and W_eff are cast to bf16 on host (halves
    input DMA, enables fast-weight-load on the PE), the GEMM accumulates in
    fp32 PSUM, and the output is stored as bf16 (halves store DMA). Host
    upcasts to fp32 and adds the bias during the unshard (exact in fp32).
    Measured rel-l2 vs fp32 reference: ~2.6e-3, far under the 2e-2 gate.
  * Whole x (6.3 MB bf16) and W (3.5 MB bf16) are SBUF-resident; x streams
    on the sync (SP) HWDGE queue while W pieces + y stores share the
    scalar (ACT) queue, so weight prefetch never queues behind stores.
  * DMA completion semaphores: a DMA's +16 arrives as 16 independent
    per-SDMA-engine +1s, so cumulative waits on a shared semaphore are
    UNSOUND with >1 DMA of that semaphore in flight (engines race ahead).
    Each DMA stream uses K=3 rotating semaphores with a producer-side
    throttle (issue of DMA j waits for DMA j-K's completion), so each
    semaphore has at most one in-flight incrementer and full-count waits
    are exact. Store completions rotate 2 sems by supertile parity (the
    next same-parity store group is data-dependent on the wait, so no
    throttle needed).
  * Supertile loop is chunk-column-major so the first m-tiles only need
    the first 512 W columns; first x supertile and the first W column
    chunk are split fine so the PE starts ~3 us in.
  * Raw-bass pipeline (explicit semaphores); PSUM->SBUF eviction (with the
    fp32->bf16 cast) on the DVE rotates through all 8 PSUM banks.
  * The first start-of-group N=512 matmul is emitted twice: this walrus
    build eats one early MATMUL (verified via the NTFF instruction
    stream); the duplicate is idempotent (start=True clears the bank).
"""

from contextlib import ExitStack

import ml_dtypes
import numpy as np

import concourse.bass as bass
import concourse.mybir as mybir
from concourse.bass_utils import run_bass_kernel_spmd

P = 128
DIM = 768
NOUT = 3 * DIM          # 2304
KT = DIM // P           # 6 k-tiles
B = 8                   # batch == n_cores
M = 64 * 64             # 4096 tokens per core
MT = M // P             # 32 m-tiles per core
TG = 512                # token supertile
NG = M // TG            # 8 supertiles
MT_G = TG // P          # 4 m-tiles per supertile

# Chunk-column processing order: the narrow 256-col chunk first, so the
# first W DMA (one group per leading chunk) is small and compute starts
# earlier. W_GROUPS must cover N_CHUNKS prefixes in this order.
N_CHUNKS = [(2048, 256), (0, 512), (512, 512), (1024, 512), (1536, 512)]
NCH = len(N_CHUNKS)     # 5 chunk columns
W_GROUPS = [(2048, 256), (0, 512), (512, 512), (1024, 1024)]  # W DMA granularity
# chunk position i -> W group that must be resident (None = already covered)
W_WAIT_OF_CHUNK = [0, 1, 2, 3, None]
N_PSUM = 8              # all psum banks
OB = 3                  # output staging buffers (supertile granularity)
KSEM = 3                # rotating DMA-completion sems per input stream

_F32 = mybir.dt.float32
_BF16 = mybir.dt.bfloat16


def _build_program():
    nc = bass.Bass()
    xt = nc.dram_tensor("xt", [P, MT, KT, P], _BF16, kind="ExternalInput")
    # piece-major W: per partition, concat over chunk-columns c of the
    # [KT, nsz_c] k-major block -> every W DMA is one contiguous run
    wt = nc.dram_tensor("wt", [P, KT * NOUT], _BF16, kind="ExternalInput")
    y = nc.dram_tensor("y", [M, NOUT], _BF16, kind="ExternalOutput")

    with ExitStack() as ctx:
        x_sb = ctx.enter_context(nc.sbuf_tensor("x_sb", [P, MT, KT, P], _BF16))
        wt_sb = ctx.enter_context(nc.sbuf_tensor("wt_sb", [P, KT, NOUT], _BF16))
        o_sb = ctx.enter_context(nc.sbuf_tensor("o_sb", [P, OB, MT_G, NOUT], _BF16))
        ps = [
            ctx.enter_context(nc.psum_tensor(f"ps{i}", [P, 512], _F32))
            for i in range(N_PSUM)
        ]
        sx = [ctx.enter_context(nc.semaphore(f"sx{i}")) for i in range(KSEM)]
        sw = [ctx.enter_context(nc.semaphore(f"sw{i}")) for i in range(len(W_GROUPS))]
        so = [ctx.enter_context(nc.semaphore(f"so{i}")) for i in range(OB)]
        s_mm = ctx.enter_context(nc.semaphore("s_mm"))
        s_tt = ctx.enter_context(nc.semaphore("s_tt"))
        block = ctx.enter_context(nc.Block())

        # Each HWDGE DMA carries ~2-3 us of ring/receipt overhead regardless
        # of size, so both input streams use few, large transfers.
        # x milestones: j = supertile g (8 DMAs). w milestones: 3 DMAs —
        # chunk-col 0, chunk-col 1, chunk-cols 2..4.
        def x_done(j):
            return sx[j % KSEM], 16 * (j // KSEM + 1)

        def w_done(j):
            return sw[j], 16

        @block.sync
        def _(sync):
            # j=0,1: halves of supertile 0 (small first transfer -> early
            # compute start); j=g+1: whole supertile g for g>=1.
            for j in range(NG + 1):
                if j >= KSEM:
                    sem, val = x_done(j - KSEM)
                    sync.wait_ge(sem, val)
                if j < 2:
                    lo, hi = 2 * j, 2 * (j + 1)
                else:
                    g = j - 1
                    # Just-in-time: don't fight the W pieces for HBM
                    # bandwidth during the ramp. Supertile g is needed
                    # one full supertile (~26 us) after this fires.
                    sync.wait_ge(s_tt, max(NCH * MT_G * (g - 2), 0) + 1)
                    lo, hi = MT_G * g, MT_G * (g + 1)
                sync.dma_start(out=x_sb[:, lo:hi], in_=xt[:, lo:hi]).then_inc(
                    sx[j % KSEM], 16
                )

        @block.scalar
        def _(scalar):
            off = 0
            for j, (n0, nsz) in enumerate(W_GROUPS):
                scalar.dma_start(
                    out=wt_sb[:, :, n0 : n0 + nsz],
                    in_=wt[:, off : off + KT * nsz],
                ).then_inc(sw[j], 16)
                off += KT * nsz
            # y stores (behind the W pieces on the same FIFO queue).
            for g in range(NG):
                for ms in range(MT_G):
                    ma = MT_G * g + ms
                    # all NCH chunk-columns of this m-tile evicted
                    if g == NG - 1:
                        scalar.wait_ge(s_tt, NCH * MT_G * g + NCH * (ms + 1))
                    else:
                        scalar.wait_ge(
                            s_tt, NCH * MT_G * g + (NCH - 1) * MT_G + ms + 1
                        )
                    scalar.dma_start(
                        out=y[ma * P : (ma + 1) * P, :], in_=o_sb[:, g % OB, ms, :]
                    ).then_inc(so[g % OB], 16)

        def chunk_order(g):
            # chunk-column-major while W streams in; m-tile-major on the
            # last supertile so its stores overlap the remaining compute
            if g == NG - 1:
                return [(c, ms) for ms in range(MT_G) for c in range(NCH)]
            return [(c, ms) for c in range(NCH) for ms in range(MT_G)]

        @block.tensor
        def _(tensor):
            # HAM warmup: keep the PE busy during the initial DMA ramp so
            # the clock gate is at 8/8 when the real stream starts. Junk
            # inputs; output bank is cleared later by a start=True group.
            for _ in range(42):
                nc.tensor.matmul(
                    ps[N_PSUM - 1][:, :128],
                    lhsT=x_sb[:, 0, 0, :],
                    rhs=wt_sb[:, 0, 0:128],
                    start=True,
                    stop=True,
                    skip_group_check=True,
                )
            cyc = 0
            for g in range(NG):
                for c, ms in chunk_order(g):
                    n0, nsz = N_CHUNKS[c]
                    if True:
                        ma = MT_G * g + ms
                        if g == 0:
                            if c == 0:
                                # half-supertile ms//2 of x loaded
                                sem, val = x_done(ms // 2)
                                tensor.wait_ge(sem, val)
                            if ms == 0 and W_WAIT_OF_CHUNK[c] is not None:
                                sem, val = w_done(W_WAIT_OF_CHUNK[c])
                                tensor.wait_ge(sem, val)
                        elif cyc % (NCH * MT_G) == 0:
                            # supertile g's x loaded (milestone j = g+1)
                            sem, val = x_done(g + 1)
                            tensor.wait_ge(sem, val)
                        if cyc >= N_PSUM:
                            # DVE finished reading this psum bank
                            tensor.wait_ge(s_tt, cyc - N_PSUM + 1)
                        for k in range(KT):
                            if g == 0 and c <= 1 and ms == 0 and k == 0:
                                # Sacrificial duplicate of the k=0 matmul:
                                # this walrus build eats one early
                                # start-of-group MATMUL (observed in the
                                # NTFF stream across variants). start=True
                                # clears the bank, so whichever copy
                                # survives the result is correct. Emitted
                                # for the first two chunk-columns to cover
                                # both the N=256 and N=512 leading groups.
                                nc.tensor.matmul(
                                    ps[cyc % N_PSUM][:, :nsz],
                                    lhsT=x_sb[:, 0, 0, :],
                                    rhs=wt_sb[:, 0, n0 : n0 + nsz],
                                    start=True,
                                    stop=False,
                                    skip_group_check=True,
                                )
                            mm = nc.tensor.matmul(
                                ps[cyc % N_PSUM][:, :nsz],
                                lhsT=x_sb[:, ma, k, :],
                                rhs=wt_sb[:, k, n0 : n0 + nsz],
                                start=(k == 0),
                                stop=(k == KT - 1),
                            )
                        mm.then_inc(s_mm, 1)
                        cyc += 1

        @block.vector
        def _(vector):
            cyc = 0
            for g in range(NG):
                for i, (c, ms) in enumerate(chunk_order(g)):
                    n0, nsz = N_CHUNKS[c]
                    vector.wait_ge(s_mm, cyc + 1)
                    if i == 0 and g >= OB:
                        # staging buffer free once supertile g-OB stored
                        vector.wait_ge(so[g % OB], 16 * MT_G * (g // OB))
                    nc.vector.tensor_copy(
                        o_sb[:, g % OB, ms, n0 : n0 + nsz],
                        ps[cyc % N_PSUM][:, :nsz],
                    ).then_inc(s_tt, 1)
                    cyc += 1

    return nc


def _prepare_inputs(x, Wqkv, bqkv, Aq, Bq, Ak, Bk, Av, Bv):
    x = np.asarray(x, dtype=np.float32)
    Wqkv = np.asarray(Wqkv, dtype=np.float32)

    # Fold LoRA: W_eff[j-th slice] = Wqkv[j-th slice] + B_j @ A_j
    w_eff = Wqkv.copy()
    for j, (A, Bm) in enumerate(((Aq, Bq), (Ak, Bk), (Av, Bv))):
        A = np.asarray(A, dtype=np.float32)
        Bm = np.asarray(Bm, dtype=np.float32)
        w_eff[j * DIM : (j + 1) * DIM] += Bm @ A

    # wt3[p, k, n] = W_eff[n, k*128+p]; DRAM is group-major: per partition,
    # concat over the W DMA groups of the k-major [KT, nsz_g] block, so every
    # W DMA reads one contiguous run per partition in SBUF iteration order.
    wt3 = w_eff.reshape(NOUT, KT, P).transpose(2, 1, 0)
    wt = np.ascontiguousarray(
        np.concatenate(
            [wt3[:, :, n0 : n0 + nsz].reshape(P, KT * nsz) for n0, nsz in W_GROUPS],
            axis=1,
        ).astype(ml_dtypes.bfloat16)
    )

    in_maps = []
    for b in range(B):
        # xt[p, ma, k, t] = x[b, ma*128+t, k*128+p], bf16
        xb = x[b].reshape(MT, P, KT, P)
        xtb = np.ascontiguousarray(
            xb.transpose(3, 0, 2, 1).astype(ml_dtypes.bfloat16)
        )
        in_maps.append({"xt": xtb, "wt": wt})
    return in_maps


def _run(inputs, trace=False, trace_kwargs=None):
    nc = _build_program()
    in_maps = _prepare_inputs(**inputs)
    res = run_bass_kernel_spmd(
        nc,
        in_maps,
        core_ids=list(range(B)),
        trace=trace,
        **(trace_kwargs or {}),
    )
    bqkv = np.asarray(inputs["bqkv"], dtype=np.float32)
    outs = res.results
    y = np.stack(
        [
            np.asarray(outs[b]["y"]).astype(np.float32).reshape(64, 64, NOUT) + bqkv
            for b in range(B)
        ]
    )
    return y, res


def kernel(**inputs):
    y, _ = _run(inputs, trace=False)
    return y
